# revision 30
# baseline (speedup 1.0000x reference)
"""Trainium2 Bass kernel for nn_NewSepConv (per-pixel separable conv, K=17).

out[b,c,h,w] = sum_{u,v} pad[b,c,h+u,w+v] * vers[b,u,h,w] * hors[b,v,h,w]
where pad = replication-pad(imgs, 8).

Decomposition (per batch b, output column w):
  E[h, c, v] = sum_r pad[b, c, r, w+v] * C_w[r, h]          (TensorE matmul)
      with band matrix C_w[r, h] = vers[b, r-h, h, w] (0 <= r-h < 17)
  out[b, c, h, w] = sum_v hors[b, v, h, w] * E[h, c, v]      (VectorE mult+reduce)

The band matrices are pre-expanded on the host (vers rearranged with zeros)
so the TensorE consumes them as ordinary dense lhsT weights.

Sharding: 8 cores, each takes a 32-column w-chunk (all batches, all rows).
h is tiled into 3 row-tiles (112, 112, 32) so each output row's full
r-contraction (h..h+16 over 272 padded rows) fits in one 128-partition matmul.
"""

import numpy as np

import concourse.mybir as mybir
import concourse.tile as tile
from concourse import bacc
from concourse.bass_utils import run_bass_kernel_spmd

F32 = mybir.dt.float32
F16 = mybir.dt.float16

B, C, H, W = 4, 3, 256, 256
K = 17
PAD = 8
NCORES = 8
WCHUNK = W // NCORES  # 32

H0 = (0, 112, 224)      # h-tile starts
M_T = (112, 112, 32)    # h-tile sizes (matmul M)
K_T = (128, 128, 48)    # r-rows per h-tile (matmul K): M_T + 16

_CACHE = {}


def _build_nc():
    nc = bacc.Bacc("TRN2", target_bir_lowering=False, debug=False)
    padk = nc.dram_tensor("padk", [B, 272, C, WCHUNK + 16], F16, kind="ExternalInput").ap()
    bandks = [
        nc.dram_tensor("bandk0", [B, 128, WCHUNK, 128], F16, kind="ExternalInput").ap(),
        nc.dram_tensor("bandk1", [B, 128, WCHUNK, 128], F16, kind="ExternalInput").ap(),
        nc.dram_tensor("bandk2", [B, 48, WCHUNK, 32], F16, kind="ExternalInput").ap(),
    ]
    horsk = nc.dram_tensor("horsk", [B, 3, 112, WCHUNK, C, K], F16, kind="ExternalInput").ap()
    outk = nc.dram_tensor("outk", [B, 3, 112, WCHUNK, C], F16, kind="ExternalOutput").ap()

    MC = (128, 128, 32)  # lhsT columns (m) incl. zero-padding to 128 for FWL

    with tile.TileContext(nc) as tc:
        with tc.tile_pool(name="pads", bufs=3) as pad_pool, \
             tc.tile_pool(name="bands", bufs=3) as band_pool, \
             tc.tile_pool(name="hors", bufs=3) as hors_pool, \
             tc.tile_pool(name="ctmp", bufs=3) as c_pool, \
             tc.tile_pool(name="mtmp", bufs=3) as m_pool, \
             tc.tile_pool(name="obuf", bufs=2) as o_pool, \
             tc.tile_pool(name="psum", bufs=2, space="PSUM") as psum_pool:
            for b in range(B):
                pad_t, band_t, hors_t = [], [], []
                for t in range(3):
                    pt = pad_pool.tile([K_T[t], C, WCHUNK + 16], F16, tag=f"pad{t}")
                    nc.sync.dma_start(out=pt[:], in_=padk[b, H0[t]:H0[t] + K_T[t]])
                    pad_t.append(pt)
                    bt = band_pool.tile([K_T[t], WCHUNK, MC[t]], F16, tag=f"band{t}")
                    if b == 0 and t == 0:
                        nc.sync.dma_start(out=bt[:, :8], in_=bandks[t][b, :, :8])
                        nc.sync.dma_start(out=bt[:, 8:], in_=bandks[t][b, :, 8:])
                    else:
                        nc.sync.dma_start(out=bt[:], in_=bandks[t][b])
                    band_t.append(bt)
                    ht = hors_pool.tile([112, WCHUNK, C, K], F16, tag=f"hors{t}")
                    nc.sync.dma_start(out=ht[:], in_=horsk[b, t])
                    hors_t.append(ht)
                ob = o_pool.tile([112, 3, WCHUNK, C], F16, tag="ob")
                nc.gpsimd.memset(ob[:, 2], 0.0)
                ct = [c_pool.tile([M_T[t], WCHUNK, C, K], F16, tag=f"c{t}",
                                  name=f"c{t}")
                      for t in range(3)]
                mt = [m_pool.tile([M_T[t], WCHUNK, C, K], F16, tag=f"m{t}",
                                  name=f"m{t}")
                      for t in range(3)]
                for wg in range(WCHUNK // 8):
                    ps = [psum_pool.tile([MC[t], 8, C, K], F32, tag=f"ps{t}",
                                         name=f"ps{t}")
                          for t in range(3)]
                    for wl8 in range(8):
                        wl = wg * 8 + wl8
                        for t in range(3):
                            nc.tensor.matmul(
                                out=ps[t][:, wl8],
                                lhsT=band_t[t][:, wl, :],
                                rhs=pad_t[t][:, :, wl:wl + K],
                                start=True, stop=True,
                            )
                    for t in range(3):
                        nc.scalar.copy(out=ct[t][:, wg * 8:(wg + 1) * 8],
                                       in_=ps[t][:M_T[t]])
                for t in range(3):
                    nc.gpsimd.tensor_tensor(out=mt[t][:], in0=ct[t][:],
                                            in1=hors_t[t][:M_T[t]],
                                            op=mybir.AluOpType.mult)
                with nc.allow_low_precision("fp16 v-sum, 17 terms"):
                    for t in range(3):
                        nc.vector.tensor_reduce(
                            out=ob[:M_T[t], t], in_=mt[t][:],
                            axis=mybir.AxisListType.X, op=mybir.AluOpType.add)
                nc.scalar.dma_start(out=outk[b].transpose([1, 0, 2, 3]), in_=ob[:])
    nc.compile()
    return nc


def _host_prep(imgs, vers, hors):
    """Build per-core input maps. Returns list of 8 dicts."""
    imgs = np.asarray(imgs, dtype=np.float32)
    vers = np.asarray(vers, dtype=np.float32)
    hors = np.asarray(hors, dtype=np.float32)

    pad_full = np.pad(imgs, ((0, 0), (0, 0), (PAD, PAD), (PAD, PAD)), mode="edge")
    pad_r = np.ascontiguousarray(pad_full.transpose(0, 2, 1, 3))  # [B, 272, C, 272]

    # band_all[t]: [B, 128, 112, W] = vers[b, p-m, h0+m, w] (zeros outside band)
    p_idx = np.arange(128)[:, None]
    m_idx = np.arange(112)[None, :]
    u = p_idx - m_idx
    u_ok = (u >= 0) & (u < K)
    uc = np.clip(u, 0, K - 1)
    band_all = []
    for t in range(3):
        h = np.clip(H0[t] + m_idx, 0, H - 1)
        valid = u_ok & (m_idx < M_T[t]) & (p_idx < K_T[t]) & (H0[t] + m_idx < H)
        h_grid = np.broadcast_to(h, (128, 112))
        a = vers[:, uc, h_grid, :]          # [B, 128, 112, W]
        a = a * valid[None, :, :, None]
        band_all.append(a.astype(np.float32))

    hors_r = hors.transpose(0, 2, 3, 1)     # [B, H, W, K]

    in_maps = []
    for k in range(NCORES):
        w0 = k * WCHUNK
        padk = np.ascontiguousarray(
            pad_r[:, :, :, w0:w0 + WCHUNK + 16].astype(np.float16))
        bks = {}
        for t in range(3):
            mc = 128 if t < 2 else M_T[t]
            bk = np.zeros((B, K_T[t], WCHUNK, mc), np.float16)
            bk[:, :, :, :M_T[t]] = band_all[t][:, :K_T[t], :M_T[t],
                                               w0:w0 + WCHUNK].transpose(0, 1, 3, 2)
            bks[f"bandk{t}"] = bk
        hk = np.zeros((B, 3, 112, WCHUNK, C, K), np.float16)
        for t in range(3):
            hk[:, t, :M_T[t]] = hors_r[:, H0[t]:H0[t] + M_T[t],
                                       w0:w0 + WCHUNK, None, :].astype(np.float16)
        in_maps.append({"padk": padk, "horsk": np.ascontiguousarray(hk), **bks})
    return in_maps


def _get_dispatch():
    """Build a pre-sharded SPMD dispatcher. Feeding already-sharded device
    arrays avoids jax resharding programs (whose neuronx-cc compile OOMs on
    large inputs)."""
    if "dispatch" in _CACHE:
        return _CACHE["dispatch"]
    import jax
    from jax.experimental.shard_map import shard_map
    from jax.sharding import Mesh, NamedSharding, PartitionSpec
    from concourse import bass2jax

    nc = _CACHE["nc"]
    bass2jax.install_neuronx_cc_hook()
    partition_name = (nc.partition_id_tensor.name
                      if nc.partition_id_tensor else None)
    in_names, out_names, out_avals = [], [], []
    for alloc in nc.m.functions[0].allocations:
        if not isinstance(alloc, mybir.MemoryLocationSet):
            continue
        name = alloc.memorylocations[0].name
        if alloc.kind == "ExternalInput":
            if name != partition_name:
                in_names.append(name)
        elif alloc.kind == "ExternalOutput":
            out_avals.append(jax.core.ShapedArray(tuple(alloc.tensor_shape),
                                                  mybir.dt.np(alloc.dtype)))
            out_names.append(name)
    n_params, n_outs = len(in_names), len(out_names)
    all_in_names = list(in_names) + list(out_names)
    if partition_name is not None:
        all_in_names.append(partition_name)
    all_in_names = tuple(all_in_names)

    def _body(*args):
        operands = list(args)
        if partition_name is not None:
            operands.append(bass2jax.partition_id_tensor())
        outs = bass2jax._bass_exec_p.bind(
            *operands,
            out_avals=tuple(out_avals),
            in_names=all_in_names,
            out_names=tuple(out_names),
            lowering_input_output_aliases=(),
            sim_require_finite=True,
            sim_require_nnan=True,
            nc=nc,
        )
        return tuple(outs)

    devices = jax.devices()[:NCORES]
    mesh = Mesh(np.asarray(devices), ("core",))
    sharding = NamedSharding(mesh, PartitionSpec("core"))
    fn = jax.jit(
        shard_map(_body, mesh=mesh,
                  in_specs=(PartitionSpec("core"),) * (n_params + n_outs),
                  out_specs=(PartitionSpec("core"),) * n_outs,
                  check_rep=False),
        donate_argnums=tuple(range(n_params, n_params + n_outs)),
        keep_unused=True)

    def make_global(shards):
        s0 = shards[0].shape
        arrs = [jax.device_put(shards[c], devices[c]) for c in range(NCORES)]
        return jax.make_array_from_single_device_arrays(
            (NCORES * s0[0], *s0[1:]), sharding, arrs)

    def dispatch(in_maps):
        gin = [make_global([m[name] for m in in_maps]) for name in in_names]
        gzero = [make_global([np.zeros(av.shape, av.dtype) for _ in range(NCORES)])
                 for av in out_avals]
        outs = fn(*gin, *gzero)
        host = [np.asarray(o) for o in outs]
        return [
            {name: host[i].reshape(NCORES, *out_avals[i].shape)[c]
             for i, name in enumerate(out_names)}
            for c in range(NCORES)
        ]

    _CACHE["dispatch"] = dispatch
    return dispatch


def _ntff_hook_via_ctypes(so_path="/opt/axon/libaxon_pjrt.so"):
    """Drive axon NTFF profiling via ctypes (antenv.axon_hooks is absent here)."""
    import contextlib
    import ctypes
    import sys
    try:
        lib = ctypes.CDLL(so_path)
    except OSError:
        return None
    if not hasattr(lib, "axon_start_nrt_profile"):
        return None
    lib.axon_start_nrt_profile.argtypes = [ctypes.POINTER(ctypes.c_int64),
                                           ctypes.c_size_t]
    lib.axon_start_nrt_profile.restype = ctypes.c_int64
    lib.axon_stop_nrt_profile.argtypes = [ctypes.c_char_p]
    lib.axon_stop_nrt_profile.restype = ctypes.c_int64

    @contextlib.contextmanager
    def _hook(output_dir, device_ids):
        import jax
        jax.devices()
        if device_ids:
            ids = (ctypes.c_int64 * len(device_ids))(*device_ids)
            rc = lib.axon_start_nrt_profile(ids, len(device_ids))
        else:
            rc = lib.axon_start_nrt_profile(None, 0)
        if rc != 0:
            raise RuntimeError(f"axon_start_nrt_profile rc={rc}")
        try:
            yield
        finally:
            n = lib.axon_stop_nrt_profile(str(output_dir).encode())
            if n < 0:
                raise RuntimeError(f"axon_stop_nrt_profile rc={n}")
            print(f"profile: {n} file(s) written to {output_dir}", file=sys.stderr)

    return _hook


class _Res:
    def __init__(self, results, exec_time_ns=None, trace_path=None):
        self.results = results
        self.exec_time_ns = exec_time_ns
        self.instructions_and_trace = ([], trace_path) if trace_path else None


def _run(in_maps, trace=False):
    if "nc" not in _CACHE:
        _CACHE["nc"] = _build_nc()
    dispatch = _get_dispatch()
    if not trace:
        return _Res(dispatch(in_maps))

    import glob as _glob
    import tempfile
    exec_time_ns = None
    trace_path = None
    try:
        from antenv.axon_hooks import get_axon_ntff_profile_hook
        hook = get_axon_ntff_profile_hook()
    except Exception:
        hook = None
    if hook is None:
        hook = _ntff_hook_via_ctypes()
    if hook is None:
        results = dispatch(in_maps)
        return _Res(results)
    neff_dir = tempfile.mkdtemp()
    with hook(neff_dir, [0]):
        results = dispatch(in_maps)
    try:
        import gauge.profiler
        from concourse._compat import FishPath
        from concourse.bass_utils import _process_ntff_profile
        ntffs = _glob.glob(neff_dir + "/*_body*.ntff")
        if ntffs:
            profile = gauge.profiler.Profile(
                profile_path=FishPath(neff_dir), kernel_dev_mode=True,
                profile_on_exit=False, bass_kernel=_CACHE["nc"].m,
                offline_processing=True, fname="*_body*", metadata={})
            perf = _process_ntff_profile(profile, neff_dir, _CACHE["nc"],
                                         list(range(NCORES)), [0], False, {},
                                         trace_events=False)
            exec_time_ns = perf.exec_time_ns
            if perf.insts_and_trace_path:
                trace_path = perf.insts_and_trace_path[1]
    except Exception as e:
        print(f"trace processing failed: {e!r}")
    return _Res(results, exec_time_ns, trace_path)


def _assemble(results):
    out = np.empty((B, C, H, W), np.float32)
    for k in range(NCORES):
        ok = results[k]["outk"].astype(np.float32)  # [B, 3, 112, 32, C]
        w0 = k * WCHUNK
        for t in range(3):
            out[:, :, H0[t]:H0[t] + M_T[t], w0:w0 + WCHUNK] = \
                ok[:, t, :M_T[t]].transpose(0, 3, 1, 2)
    return out


def kernel(imgs, vers, hors):
    in_maps = _host_prep(imgs, vers, hors)
    res = _run(in_maps)
    return _assemble(res.results)


def kernel_traced(imgs, vers, hors):
    """Like kernel() but returns (output, BassKernelResults) with profiling."""
    in_maps = _host_prep(imgs, vers, hors)
    res = _run(in_maps, trace=True)
    return _assemble(res.results), res



# revision 34
# speedup vs baseline: 1.0929x; 1.0929x over previous
"""Trainium2 Bass kernel for nn_NewSepConv (per-pixel separable conv, K=17).

out[b,c,h,w] = sum_{u,v} pad[b,c,h+u,w+v] * vers[b,u,h,w] * hors[b,v,h,w]
where pad = replication-pad(imgs, 8).

Decomposition (per batch b, output column w):
  E[h, c, v] = sum_r pad[b, c, r, w+v] * C_w[r, h]          (TensorE matmul)
      with band matrix C_w[r, h] = vers[b, r-h, h, w] (0 <= r-h < 17)
  out[b, c, h, w] = sum_v hors[b, v, h, w] * E[h, c, v]      (VectorE mult+reduce)

The band matrices are pre-expanded on the host (vers rearranged with zeros)
so the TensorE consumes them as ordinary dense lhsT weights.

Sharding: 8 cores, each takes a 32-column w-chunk (all batches, all rows).
h is tiled into 3 row-tiles (112, 112, 32) so each output row's full
r-contraction (h..h+16 over 272 padded rows) fits in one 128-partition matmul.
"""

import numpy as np

import concourse.mybir as mybir
import concourse.tile as tile
from concourse import bacc
from concourse.bass_utils import run_bass_kernel_spmd

F32 = mybir.dt.float32
F16 = mybir.dt.float16

B, C, H, W = 4, 3, 256, 256
K = 17
PAD = 8
NCORES = 8
WCHUNK = W // NCORES  # 32

H0 = (0, 112, 224)      # h-tile starts
M_T = (112, 112, 32)    # h-tile sizes (matmul M)
K_T = (128, 128, 48)    # r-rows per h-tile (matmul K): M_T + 16

_CACHE = {}


def _build_nc():
    nc = bacc.Bacc("TRN2", target_bir_lowering=False, debug=False)
    padk = nc.dram_tensor("padk", [B, 272, C, WCHUNK + 16], F16, kind="ExternalInput").ap()
    bandks = [
        nc.dram_tensor("bandk0", [B, 128, WCHUNK, 128], F16, kind="ExternalInput").ap(),
        nc.dram_tensor("bandk1", [B, 128, WCHUNK, 128], F16, kind="ExternalInput").ap(),
        nc.dram_tensor("bandk2", [B, 48, WCHUNK, 32], F16, kind="ExternalInput").ap(),
    ]
    horsk = nc.dram_tensor("horsk", [B, 3, 112, WCHUNK, K], F16, kind="ExternalInput").ap()
    outk = nc.dram_tensor("outk", [B, 3, 112, WCHUNK, C], F16, kind="ExternalOutput").ap()

    MC = (128, 128, 32)  # lhsT columns (m) incl. zero-padding to 128 for FWL

    with tile.TileContext(nc) as tc:
        with tc.tile_pool(name="pads", bufs=3) as pad_pool, \
             tc.tile_pool(name="bands", bufs=3) as band_pool, \
             tc.tile_pool(name="hors", bufs=3) as hors_pool, \
             tc.tile_pool(name="ctmp", bufs=3) as c_pool, \
             tc.tile_pool(name="mtmp", bufs=3) as m_pool, \
             tc.tile_pool(name="obuf", bufs=2) as o_pool, \
             tc.tile_pool(name="psum", bufs=3, space="PSUM") as psum_pool, \
             tc.tile_pool(name="psum2", bufs=2, space="PSUM") as psum2_pool:
            for b in range(B):
                pad_t, band_t, hors_t = [], [], []
                for t in range(3):
                    pt = pad_pool.tile([K_T[t], C, WCHUNK + 16], F16, tag=f"pad{t}")
                    nc.sync.dma_start(out=pt[:], in_=padk[b, H0[t]:H0[t] + K_T[t]])
                    pad_t.append(pt)
                    bt = band_pool.tile([K_T[t], WCHUNK, MC[t]], F16, tag=f"band{t}")
                    if b == 0 and t == 0:
                        nc.sync.dma_start(out=bt[:, :8], in_=bandks[t][b, :, :8])
                        nc.sync.dma_start(out=bt[:, 8:], in_=bandks[t][b, :, 8:])
                    else:
                        nc.sync.dma_start(out=bt[:], in_=bandks[t][b])
                    band_t.append(bt)
                    ht = hors_pool.tile([112, WCHUNK, K], F16, tag=f"hors{t}")
                    nc.sync.dma_start(out=ht[:], in_=horsk[b, t])
                    hors_t.append(ht)
                ob = o_pool.tile([112, 3, WCHUNK, C], F16, tag="ob")
                nc.gpsimd.memset(ob[:, 2], 0.0)
                ct = [c_pool.tile([M_T[t], WCHUNK, C, K], F16, tag=f"c{t}",
                                  name=f"c{t}")
                      for t in range(2)]
                mt = [m_pool.tile([M_T[t], WCHUNK, C, K], F16, tag=f"m{t}",
                                  name=f"m{t}")
                      for t in range(3)]
                for wg in range(WCHUNK // 8):
                    ps = [psum_pool.tile([MC[t], 8, C, K], F32, tag=f"ps{t}",
                                         name=f"ps{t}")
                          for t in range(2)]
                    ps.append(psum2_pool.tile([MC[2], 8, C, K], F32, tag="ps2",
                                              name="ps2"))
                    for wl8 in range(8):
                        wl = wg * 8 + wl8
                        for t in range(3):
                            nc.tensor.matmul(
                                out=ps[t][:, wl8],
                                lhsT=band_t[t][:, wl, :],
                                rhs=pad_t[t][:, :, wl:wl + K],
                                start=True, stop=True,
                            )
                    for t in range(2):
                        nc.scalar.copy(out=ct[t][:, wg * 8:(wg + 1) * 8],
                                       in_=ps[t][:M_T[t]])
                    hs2 = hors_t[2][:32, wg * 8:(wg + 1) * 8]
                    nc.vector.tensor_tensor(
                        out=mt[2][:, wg * 8:(wg + 1) * 8], in0=ps[2][:32],
                        in1=hs2.unsqueeze(2).broadcast_to([32, 8, C, K]),
                        op=mybir.AluOpType.mult)
                for t in range(2):
                    hb = hors_t[t][:M_T[t]].unsqueeze(2).broadcast_to(
                        [M_T[t], WCHUNK, C, K])
                    nc.gpsimd.tensor_tensor(out=mt[t][:], in0=ct[t][:], in1=hb,
                                            op=mybir.AluOpType.mult)
                with nc.allow_low_precision("fp16 v-sum, 17 terms"):
                    for t in range(3):
                        nc.vector.tensor_reduce(
                            out=ob[:M_T[t], t], in_=mt[t][:],
                            axis=mybir.AxisListType.X, op=mybir.AluOpType.add)
                nc.sync.dma_start(out=outk[b].transpose([1, 0, 2, 3]), in_=ob[:])
    nc.compile()
    return nc


def _host_prep(imgs, vers, hors):
    """Build per-core input maps. Returns list of 8 dicts."""
    imgs = np.asarray(imgs, dtype=np.float32)
    vers = np.asarray(vers, dtype=np.float32)
    hors = np.asarray(hors, dtype=np.float32)

    pad_full = np.pad(imgs, ((0, 0), (0, 0), (PAD, PAD), (PAD, PAD)), mode="edge")
    pad_r = np.ascontiguousarray(pad_full.transpose(0, 2, 1, 3))  # [B, 272, C, 272]

    # band_all[t]: [B, 128, 112, W] = vers[b, p-m, h0+m, w] (zeros outside band)
    p_idx = np.arange(128)[:, None]
    m_idx = np.arange(112)[None, :]
    u = p_idx - m_idx
    u_ok = (u >= 0) & (u < K)
    uc = np.clip(u, 0, K - 1)
    band_all = []
    for t in range(3):
        h = np.clip(H0[t] + m_idx, 0, H - 1)
        valid = u_ok & (m_idx < M_T[t]) & (p_idx < K_T[t]) & (H0[t] + m_idx < H)
        h_grid = np.broadcast_to(h, (128, 112))
        a = vers[:, uc, h_grid, :]          # [B, 128, 112, W]
        a = a * valid[None, :, :, None]
        band_all.append(a.astype(np.float32))

    hors_r = hors.transpose(0, 2, 3, 1)     # [B, H, W, K]

    in_maps = []
    for k in range(NCORES):
        w0 = k * WCHUNK
        padk = np.ascontiguousarray(
            pad_r[:, :, :, w0:w0 + WCHUNK + 16].astype(np.float16))
        bks = {}
        for t in range(3):
            mc = 128 if t < 2 else M_T[t]
            bk = np.zeros((B, K_T[t], WCHUNK, mc), np.float16)
            bk[:, :, :, :M_T[t]] = band_all[t][:, :K_T[t], :M_T[t],
                                               w0:w0 + WCHUNK].transpose(0, 1, 3, 2)
            bks[f"bandk{t}"] = bk
        hk = np.zeros((B, 3, 112, WCHUNK, K), np.float16)
        for t in range(3):
            hk[:, t, :M_T[t]] = hors_r[:, H0[t]:H0[t] + M_T[t],
                                       w0:w0 + WCHUNK, :].astype(np.float16)
        in_maps.append({"padk": padk, "horsk": np.ascontiguousarray(hk), **bks})
    return in_maps


def _get_dispatch():
    """Build a pre-sharded SPMD dispatcher. Feeding already-sharded device
    arrays avoids jax resharding programs (whose neuronx-cc compile OOMs on
    large inputs)."""
    if "dispatch" in _CACHE:
        return _CACHE["dispatch"]
    import jax
    from jax.experimental.shard_map import shard_map
    from jax.sharding import Mesh, NamedSharding, PartitionSpec
    from concourse import bass2jax

    nc = _CACHE["nc"]
    bass2jax.install_neuronx_cc_hook()
    partition_name = (nc.partition_id_tensor.name
                      if nc.partition_id_tensor else None)
    in_names, out_names, out_avals = [], [], []
    for alloc in nc.m.functions[0].allocations:
        if not isinstance(alloc, mybir.MemoryLocationSet):
            continue
        name = alloc.memorylocations[0].name
        if alloc.kind == "ExternalInput":
            if name != partition_name:
                in_names.append(name)
        elif alloc.kind == "ExternalOutput":
            out_avals.append(jax.core.ShapedArray(tuple(alloc.tensor_shape),
                                                  mybir.dt.np(alloc.dtype)))
            out_names.append(name)
    n_params, n_outs = len(in_names), len(out_names)
    all_in_names = list(in_names) + list(out_names)
    if partition_name is not None:
        all_in_names.append(partition_name)
    all_in_names = tuple(all_in_names)

    def _body(*args):
        operands = list(args)
        if partition_name is not None:
            operands.append(bass2jax.partition_id_tensor())
        outs = bass2jax._bass_exec_p.bind(
            *operands,
            out_avals=tuple(out_avals),
            in_names=all_in_names,
            out_names=tuple(out_names),
            lowering_input_output_aliases=(),
            sim_require_finite=True,
            sim_require_nnan=True,
            nc=nc,
        )
        return tuple(outs)

    devices = jax.devices()[:NCORES]
    mesh = Mesh(np.asarray(devices), ("core",))
    sharding = NamedSharding(mesh, PartitionSpec("core"))
    fn = jax.jit(
        shard_map(_body, mesh=mesh,
                  in_specs=(PartitionSpec("core"),) * (n_params + n_outs),
                  out_specs=(PartitionSpec("core"),) * n_outs,
                  check_rep=False),
        donate_argnums=tuple(range(n_params, n_params + n_outs)),
        keep_unused=True)

    def make_global(shards):
        s0 = shards[0].shape
        arrs = [jax.device_put(shards[c], devices[c]) for c in range(NCORES)]
        return jax.make_array_from_single_device_arrays(
            (NCORES * s0[0], *s0[1:]), sharding, arrs)

    def dispatch(in_maps):
        gin = [make_global([m[name] for m in in_maps]) for name in in_names]
        gzero = [make_global([np.zeros(av.shape, av.dtype) for _ in range(NCORES)])
                 for av in out_avals]
        outs = fn(*gin, *gzero)
        host = [np.asarray(o) for o in outs]
        return [
            {name: host[i].reshape(NCORES, *out_avals[i].shape)[c]
             for i, name in enumerate(out_names)}
            for c in range(NCORES)
        ]

    _CACHE["dispatch"] = dispatch
    return dispatch


def _ntff_hook_via_ctypes(so_path="/opt/axon/libaxon_pjrt.so"):
    """Drive axon NTFF profiling via ctypes (antenv.axon_hooks is absent here)."""
    import contextlib
    import ctypes
    import sys
    try:
        lib = ctypes.CDLL(so_path)
    except OSError:
        return None
    if not hasattr(lib, "axon_start_nrt_profile"):
        return None
    lib.axon_start_nrt_profile.argtypes = [ctypes.POINTER(ctypes.c_int64),
                                           ctypes.c_size_t]
    lib.axon_start_nrt_profile.restype = ctypes.c_int64
    lib.axon_stop_nrt_profile.argtypes = [ctypes.c_char_p]
    lib.axon_stop_nrt_profile.restype = ctypes.c_int64

    @contextlib.contextmanager
    def _hook(output_dir, device_ids):
        import jax
        jax.devices()
        if device_ids:
            ids = (ctypes.c_int64 * len(device_ids))(*device_ids)
            rc = lib.axon_start_nrt_profile(ids, len(device_ids))
        else:
            rc = lib.axon_start_nrt_profile(None, 0)
        if rc != 0:
            raise RuntimeError(f"axon_start_nrt_profile rc={rc}")
        try:
            yield
        finally:
            n = lib.axon_stop_nrt_profile(str(output_dir).encode())
            if n < 0:
                raise RuntimeError(f"axon_stop_nrt_profile rc={n}")
            print(f"profile: {n} file(s) written to {output_dir}", file=sys.stderr)

    return _hook


class _Res:
    def __init__(self, results, exec_time_ns=None, trace_path=None):
        self.results = results
        self.exec_time_ns = exec_time_ns
        self.instructions_and_trace = ([], trace_path) if trace_path else None


def _run(in_maps, trace=False):
    if "nc" not in _CACHE:
        _CACHE["nc"] = _build_nc()
    dispatch = _get_dispatch()
    if not trace:
        return _Res(dispatch(in_maps))

    import glob as _glob
    import tempfile
    exec_time_ns = None
    trace_path = None
    try:
        from antenv.axon_hooks import get_axon_ntff_profile_hook
        hook = get_axon_ntff_profile_hook()
    except Exception:
        hook = None
    if hook is None:
        hook = _ntff_hook_via_ctypes()
    if hook is None:
        results = dispatch(in_maps)
        return _Res(results)
    neff_dir = tempfile.mkdtemp()
    with hook(neff_dir, [0]):
        results = dispatch(in_maps)
    try:
        import gauge.profiler
        from concourse._compat import FishPath
        from concourse.bass_utils import _process_ntff_profile
        ntffs = _glob.glob(neff_dir + "/*_body*.ntff")
        if ntffs:
            profile = gauge.profiler.Profile(
                profile_path=FishPath(neff_dir), kernel_dev_mode=True,
                profile_on_exit=False, bass_kernel=_CACHE["nc"].m,
                offline_processing=True, fname="*_body*", metadata={})
            perf = _process_ntff_profile(profile, neff_dir, _CACHE["nc"],
                                         list(range(NCORES)), [0], False, {},
                                         trace_events=False)
            exec_time_ns = perf.exec_time_ns
            if perf.insts_and_trace_path:
                trace_path = perf.insts_and_trace_path[1]
    except Exception as e:
        print(f"trace processing failed: {e!r}")
    return _Res(results, exec_time_ns, trace_path)


def _assemble(results):
    out = np.empty((B, C, H, W), np.float32)
    for k in range(NCORES):
        ok = results[k]["outk"].astype(np.float32)  # [B, 3, 112, 32, C]
        w0 = k * WCHUNK
        for t in range(3):
            out[:, :, H0[t]:H0[t] + M_T[t], w0:w0 + WCHUNK] = \
                ok[:, t, :M_T[t]].transpose(0, 3, 1, 2)
    return out


def kernel(imgs, vers, hors):
    in_maps = _host_prep(imgs, vers, hors)
    res = _run(in_maps)
    return _assemble(res.results)


def kernel_traced(imgs, vers, hors):
    """Like kernel() but returns (output, BassKernelResults) with profiling."""
    in_maps = _host_prep(imgs, vers, hors)
    res = _run(in_maps, trace=True)
    return _assemble(res.results), res



# revision 37
# speedup vs baseline: 1.2917x; 1.1819x over previous
"""Trainium2 Bass kernel for nn_NewSepConv (per-pixel separable conv, K=17).

out[b,c,h,w] = sum_{u,v} pad[b,c,h+u,w+v] * vers[b,u,h,w] * hors[b,v,h,w]
where pad = replication-pad(imgs, 8).

Decomposition (per batch b, output column w):
  E[h, c, v] = sum_r pad[b, c, r, w+v] * C_w[r, h]          (TensorE matmul)
      with band matrix C_w[r, h] = vers[b, r-h, h, w] (0 <= r-h < 17)
  out[b, c, h, w] = sum_v hors[b, v, h, w] * E[h, c, v]      (VectorE mult+reduce)

The band matrices are pre-expanded on the host (vers rearranged with zeros)
so the TensorE consumes them as ordinary dense lhsT weights.

Sharding: 8 cores, each takes a 32-column w-chunk (all batches, all rows).
h is tiled into 3 row-tiles (112, 112, 32) so each output row's full
r-contraction (h..h+16 over 272 padded rows) fits in one 128-partition matmul.
"""

import numpy as np

import concourse.mybir as mybir
import concourse.tile as tile
from concourse import bacc
from concourse.bass_utils import run_bass_kernel_spmd

F32 = mybir.dt.float32
F16 = mybir.dt.float16

B, C, H, W = 4, 3, 256, 256
K = 17
PAD = 8
NCORES = 8
WCHUNK = W // NCORES  # 32

H0 = (0, 112, 224)      # h-tile starts
M_T = (112, 112, 32)    # h-tile sizes (matmul M)
K_T = (128, 128, 48)    # r-rows per h-tile (matmul K): M_T + 16

_CACHE = {}


def _build_nc():
    nc = bacc.Bacc("TRN2", target_bir_lowering=False, debug=False)
    padk = nc.dram_tensor("padk", [B, 272, C, WCHUNK + 16], F16, kind="ExternalInput").ap()
    bandks = [
        nc.dram_tensor("bandk0", [B, 128, WCHUNK, 128], F16, kind="ExternalInput").ap(),
        nc.dram_tensor("bandk1", [B, 128, WCHUNK, 128], F16, kind="ExternalInput").ap(),
        nc.dram_tensor("bandk2", [B, 48, WCHUNK, 32], F16, kind="ExternalInput").ap(),
    ]
    horsk = nc.dram_tensor("horsk", [B, 3, 112, WCHUNK, K], F16, kind="ExternalInput").ap()
    outk = nc.dram_tensor("outk", [B, 3, 112, WCHUNK, C], F16, kind="ExternalOutput").ap()

    MC = (128, 128, 32)  # lhsT columns (m) incl. zero-padding to 128 for FWL

    with tile.TileContext(nc) as tc:
        with tc.tile_pool(name="pads", bufs=3) as pad_pool, \
             tc.tile_pool(name="bands", bufs=3) as band_pool, \
             tc.tile_pool(name="hors", bufs=3) as hors_pool, \
             tc.tile_pool(name="ctmp", bufs=3) as c_pool, \
             tc.tile_pool(name="mtmp", bufs=3) as m_pool, \
             tc.tile_pool(name="obuf", bufs=2) as o_pool, \
             tc.tile_pool(name="psum", bufs=2, space="PSUM") as psum_pool:
            for b in range(B):
                pad_t, band_t, hors_t = [], [], []
                for t in range(3):
                    pt = pad_pool.tile([K_T[t], C, WCHUNK + 16], F16, tag=f"pad{t}")
                    nc.sync.dma_start(out=pt[:], in_=padk[b, H0[t]:H0[t] + K_T[t]])
                    pad_t.append(pt)
                    bt = band_pool.tile([K_T[t], WCHUNK, MC[t]], F16, tag=f"band{t}")
                    if b == 0 and t == 0:
                        nc.sync.dma_start(out=bt[:, :8], in_=bandks[t][b, :, :8])
                        nc.sync.dma_start(out=bt[:, 8:], in_=bandks[t][b, :, 8:])
                    else:
                        nc.sync.dma_start(out=bt[:], in_=bandks[t][b])
                    band_t.append(bt)
                    ht = hors_pool.tile([112, WCHUNK, K], F16, tag=f"hors{t}")
                    nc.sync.dma_start(out=ht[:], in_=horsk[b, t])
                    hors_t.append(ht)
                ob = o_pool.tile([112, 3, WCHUNK, C], F16, tag="ob")
                nc.gpsimd.memset(ob[:, 2], 0.0)
                ct = [c_pool.tile([M_T[t], WCHUNK, C, K], F16, tag=f"c{t}",
                                  name=f"c{t}")
                      for t in range(2)]
                mt = [m_pool.tile([M_T[t], WCHUNK, C, K], F16, tag=f"m{t}",
                                  name=f"m{t}")
                      for t in range(3)]
                for wg in range(WCHUNK // 8):
                    ps = [psum_pool.tile([MC[t], 8, C, K], F32, tag=f"ps{t}",
                                         name=f"ps{t}")
                          for t in range(3)]
                    for wl8 in range(8):
                        wl = wg * 8 + wl8
                        for t in range(3):
                            nc.tensor.matmul(
                                out=ps[t][:, wl8],
                                lhsT=band_t[t][:, wl, :],
                                rhs=pad_t[t][:, :, wl:wl + K],
                                start=True, stop=True,
                            )
                    for t in range(2):
                        nc.scalar.copy(out=ct[t][:, wg * 8:(wg + 1) * 8],
                                       in_=ps[t][:M_T[t]])
                    hs2 = hors_t[2][:32, wg * 8:(wg + 1) * 8]
                    nc.vector.tensor_tensor(
                        out=mt[2][:, wg * 8:(wg + 1) * 8], in0=ps[2][:32],
                        in1=hs2.unsqueeze(2).broadcast_to([32, 8, C, K]),
                        op=mybir.AluOpType.mult)
                for t in range(2):
                    hb = hors_t[t][:M_T[t]].unsqueeze(2).broadcast_to(
                        [M_T[t], WCHUNK, C, K])
                    nc.gpsimd.tensor_tensor(out=mt[t][:], in0=ct[t][:], in1=hb,
                                            op=mybir.AluOpType.mult)
                with nc.allow_low_precision("fp16 v-sum, 17 terms"):
                    for t in range(3):
                        nc.vector.tensor_reduce(
                            out=ob[:M_T[t], t], in_=mt[t][:],
                            axis=mybir.AxisListType.X, op=mybir.AluOpType.add)
                nc.scalar.dma_start(out=outk[b].transpose([1, 0, 2, 3]), in_=ob[:])
    nc.compile()
    return nc


def _host_prep(imgs, vers, hors):
    """Build per-core input maps. Returns list of 8 dicts."""
    imgs = np.asarray(imgs, dtype=np.float32)
    vers = np.asarray(vers, dtype=np.float32)
    hors = np.asarray(hors, dtype=np.float32)

    pad_full = np.pad(imgs, ((0, 0), (0, 0), (PAD, PAD), (PAD, PAD)), mode="edge")
    pad_r = np.ascontiguousarray(pad_full.transpose(0, 2, 1, 3))  # [B, 272, C, 272]

    # band_all[t]: [B, 128, 112, W] = vers[b, p-m, h0+m, w] (zeros outside band)
    p_idx = np.arange(128)[:, None]
    m_idx = np.arange(112)[None, :]
    u = p_idx - m_idx
    u_ok = (u >= 0) & (u < K)
    uc = np.clip(u, 0, K - 1)
    band_all = []
    for t in range(3):
        h = np.clip(H0[t] + m_idx, 0, H - 1)
        valid = u_ok & (m_idx < M_T[t]) & (p_idx < K_T[t]) & (H0[t] + m_idx < H)
        h_grid = np.broadcast_to(h, (128, 112))
        a = vers[:, uc, h_grid, :]          # [B, 128, 112, W]
        a = a * valid[None, :, :, None]
        band_all.append(a.astype(np.float32))

    hors_r = hors.transpose(0, 2, 3, 1)     # [B, H, W, K]

    in_maps = []
    for k in range(NCORES):
        w0 = k * WCHUNK
        padk = np.ascontiguousarray(
            pad_r[:, :, :, w0:w0 + WCHUNK + 16].astype(np.float16))
        bks = {}
        for t in range(3):
            mc = 128 if t < 2 else M_T[t]
            bk = np.zeros((B, K_T[t], WCHUNK, mc), np.float16)
            bk[:, :, :, :M_T[t]] = band_all[t][:, :K_T[t], :M_T[t],
                                               w0:w0 + WCHUNK].transpose(0, 1, 3, 2)
            bks[f"bandk{t}"] = bk
        hk = np.zeros((B, 3, 112, WCHUNK, K), np.float16)
        for t in range(3):
            hk[:, t, :M_T[t]] = hors_r[:, H0[t]:H0[t] + M_T[t],
                                       w0:w0 + WCHUNK, :].astype(np.float16)
        in_maps.append({"padk": padk, "horsk": np.ascontiguousarray(hk), **bks})
    return in_maps


def _get_dispatch():
    """Build a pre-sharded SPMD dispatcher. Feeding already-sharded device
    arrays avoids jax resharding programs (whose neuronx-cc compile OOMs on
    large inputs)."""
    if "dispatch" in _CACHE:
        return _CACHE["dispatch"]
    import jax
    from jax.experimental.shard_map import shard_map
    from jax.sharding import Mesh, NamedSharding, PartitionSpec
    from concourse import bass2jax

    nc = _CACHE["nc"]
    bass2jax.install_neuronx_cc_hook()
    partition_name = (nc.partition_id_tensor.name
                      if nc.partition_id_tensor else None)
    in_names, out_names, out_avals = [], [], []
    for alloc in nc.m.functions[0].allocations:
        if not isinstance(alloc, mybir.MemoryLocationSet):
            continue
        name = alloc.memorylocations[0].name
        if alloc.kind == "ExternalInput":
            if name != partition_name:
                in_names.append(name)
        elif alloc.kind == "ExternalOutput":
            out_avals.append(jax.core.ShapedArray(tuple(alloc.tensor_shape),
                                                  mybir.dt.np(alloc.dtype)))
            out_names.append(name)
    n_params, n_outs = len(in_names), len(out_names)
    all_in_names = list(in_names) + list(out_names)
    if partition_name is not None:
        all_in_names.append(partition_name)
    all_in_names = tuple(all_in_names)

    def _body(*args):
        operands = list(args)
        if partition_name is not None:
            operands.append(bass2jax.partition_id_tensor())
        outs = bass2jax._bass_exec_p.bind(
            *operands,
            out_avals=tuple(out_avals),
            in_names=all_in_names,
            out_names=tuple(out_names),
            lowering_input_output_aliases=(),
            sim_require_finite=True,
            sim_require_nnan=True,
            nc=nc,
        )
        return tuple(outs)

    devices = jax.devices()[:NCORES]
    mesh = Mesh(np.asarray(devices), ("core",))
    sharding = NamedSharding(mesh, PartitionSpec("core"))
    fn = jax.jit(
        shard_map(_body, mesh=mesh,
                  in_specs=(PartitionSpec("core"),) * (n_params + n_outs),
                  out_specs=(PartitionSpec("core"),) * n_outs,
                  check_rep=False),
        donate_argnums=tuple(range(n_params, n_params + n_outs)),
        keep_unused=True)

    def make_global(shards):
        s0 = shards[0].shape
        arrs = [jax.device_put(shards[c], devices[c]) for c in range(NCORES)]
        return jax.make_array_from_single_device_arrays(
            (NCORES * s0[0], *s0[1:]), sharding, arrs)

    def dispatch(in_maps):
        gin = [make_global([m[name] for m in in_maps]) for name in in_names]
        gzero = [make_global([np.zeros(av.shape, av.dtype) for _ in range(NCORES)])
                 for av in out_avals]
        outs = fn(*gin, *gzero)
        host = [np.asarray(o) for o in outs]
        return [
            {name: host[i].reshape(NCORES, *out_avals[i].shape)[c]
             for i, name in enumerate(out_names)}
            for c in range(NCORES)
        ]

    _CACHE["dispatch"] = dispatch
    return dispatch


def _ntff_hook_via_ctypes(so_path="/opt/axon/libaxon_pjrt.so"):
    """Drive axon NTFF profiling via ctypes (antenv.axon_hooks is absent here)."""
    import contextlib
    import ctypes
    import sys
    try:
        lib = ctypes.CDLL(so_path)
    except OSError:
        return None
    if not hasattr(lib, "axon_start_nrt_profile"):
        return None
    lib.axon_start_nrt_profile.argtypes = [ctypes.POINTER(ctypes.c_int64),
                                           ctypes.c_size_t]
    lib.axon_start_nrt_profile.restype = ctypes.c_int64
    lib.axon_stop_nrt_profile.argtypes = [ctypes.c_char_p]
    lib.axon_stop_nrt_profile.restype = ctypes.c_int64

    @contextlib.contextmanager
    def _hook(output_dir, device_ids):
        import jax
        jax.devices()
        if device_ids:
            ids = (ctypes.c_int64 * len(device_ids))(*device_ids)
            rc = lib.axon_start_nrt_profile(ids, len(device_ids))
        else:
            rc = lib.axon_start_nrt_profile(None, 0)
        if rc != 0:
            raise RuntimeError(f"axon_start_nrt_profile rc={rc}")
        try:
            yield
        finally:
            n = lib.axon_stop_nrt_profile(str(output_dir).encode())
            if n < 0:
                raise RuntimeError(f"axon_stop_nrt_profile rc={n}")
            print(f"profile: {n} file(s) written to {output_dir}", file=sys.stderr)

    return _hook


class _Res:
    def __init__(self, results, exec_time_ns=None, trace_path=None):
        self.results = results
        self.exec_time_ns = exec_time_ns
        self.instructions_and_trace = ([], trace_path) if trace_path else None


def _run(in_maps, trace=False):
    if "nc" not in _CACHE:
        _CACHE["nc"] = _build_nc()
    dispatch = _get_dispatch()
    if not trace:
        return _Res(dispatch(in_maps))

    import glob as _glob
    import tempfile
    exec_time_ns = None
    trace_path = None
    try:
        from antenv.axon_hooks import get_axon_ntff_profile_hook
        hook = get_axon_ntff_profile_hook()
    except Exception:
        hook = None
    if hook is None:
        hook = _ntff_hook_via_ctypes()
    if hook is None:
        results = dispatch(in_maps)
        return _Res(results)
    neff_dir = tempfile.mkdtemp()
    with hook(neff_dir, [0]):
        results = dispatch(in_maps)
    try:
        import gauge.profiler
        from concourse._compat import FishPath
        from concourse.bass_utils import _process_ntff_profile
        ntffs = _glob.glob(neff_dir + "/*_body*.ntff")
        if ntffs:
            profile = gauge.profiler.Profile(
                profile_path=FishPath(neff_dir), kernel_dev_mode=True,
                profile_on_exit=False, bass_kernel=_CACHE["nc"].m,
                offline_processing=True, fname="*_body*", metadata={})
            perf = _process_ntff_profile(profile, neff_dir, _CACHE["nc"],
                                         list(range(NCORES)), [0], False, {},
                                         trace_events=False)
            exec_time_ns = perf.exec_time_ns
            if perf.insts_and_trace_path:
                trace_path = perf.insts_and_trace_path[1]
    except Exception as e:
        print(f"trace processing failed: {e!r}")
    return _Res(results, exec_time_ns, trace_path)


def _assemble(results):
    out = np.empty((B, C, H, W), np.float32)
    for k in range(NCORES):
        ok = results[k]["outk"].astype(np.float32)  # [B, 3, 112, 32, C]
        w0 = k * WCHUNK
        for t in range(3):
            out[:, :, H0[t]:H0[t] + M_T[t], w0:w0 + WCHUNK] = \
                ok[:, t, :M_T[t]].transpose(0, 3, 1, 2)
    return out


def kernel(imgs, vers, hors):
    in_maps = _host_prep(imgs, vers, hors)
    res = _run(in_maps)
    return _assemble(res.results)


def kernel_traced(imgs, vers, hors):
    """Like kernel() but returns (output, BassKernelResults) with profiling."""
    in_maps = _host_prep(imgs, vers, hors)
    res = _run(in_maps, trace=True)
    return _assemble(res.results), res

